# revision 1
# baseline (speedup 1.0000x reference)
"""GQA (16 q-heads / 4 KV groups, S=4096, D=1024, causal) on 8 TRN2 NeuronCores.

Sharding: tensor-parallel over query heads — 2 q-heads + their KV group per
core. wq/wk/wv column-sharded, wo row-sharded; the 8 partial outputs are
summed on the host (no device collectives needed).

Per-core program (all matmuls bf16, f32 PSUM accumulation):
  qT   = (wq_c @ x^T)            [128, 4096]   (2 heads x 64 dims, transposed)
  kvT  = (wkv_c @ x^T)           [128, 4096]   (rows 0-63 kT, 64-127 vT)
  v    = transpose(vT) + ones col               (DMA transpose, [128,65] tiles)
  per (q-chunk qc of 512, head h):
    sT[k,q] = kT_kt^T . qT_h     (K=64 matmuls, PSUM [128,3,512] groups)
    pT = exp(sT/8)               (one ACT inst per 3-tile group)
    causal mask on diagonal strips (DVE memset + tri-mask mul)
    ctxT[65,512] += v_aug_kt^T . pT   (row 64 = softmax denominators)
    denominators -> DRAM -> [128,4] -> reciprocal -> DRAM -> broadcast
    ctxT normalized in-place (DVE), then out rows = ctxT_chunk^T @ woT
Softmax uses no max-subtraction: s/8 ~ N(0,1), max ~ 10 -> exp safe in f32.
"""

import numpy as np
import ml_dtypes

BF16 = ml_dtypes.bfloat16

S = 4096
DIN = 1024
DIM = 1024
NH, NKV, HD = 16, 4, 64
NCORES = 8
QC = 512          # q chunk width
NQC = S // QC     # 8
NKT = S // 128    # 32 k tiles
GROUP = 3         # k-tiles per ACT exp instruction (3 PSUM banks)

_CACHE = {}


def _build_nc(debug=False):
    import concourse.bass as bass
    import concourse.mybir as mybir
    import concourse.tile as tile
    from concourse import bacc
    from concourse.tile_rust import add_dep_helper
    from contextlib import ExitStack

    fp32 = mybir.dt.float32
    bf16 = mybir.dt.bfloat16
    Exp = mybir.ActivationFunctionType.Exp

    nc = bacc.Bacc()
    xT_d = nc.dram_tensor("xT", [DIN, S], bf16, kind="ExternalInput")
    wqT_d = nc.dram_tensor("wqT", [DIN, 128], bf16, kind="ExternalInput")
    wkvT_d = nc.dram_tensor("wkvT", [DIN, 128], bf16, kind="ExternalInput")
    woT_d = nc.dram_tensor("woT", [128, DIM], bf16, kind="ExternalInput")
    mask_d = nc.dram_tensor("trimask", [128, 128], bf16, kind="ExternalInput")
    out_d = nc.dram_tensor("out", [S, DIM], fp32, kind="ExternalOutput")
    skind = {"kind": "ExternalOutput"} if debug else {}
    sums_d = nc.dram_tensor("sums_scratch", [2, S], fp32, **skind)
    rec_d = nc.dram_tensor("recips_scratch", [2, S], bf16, **skind)
    if debug:
        dbg_qT = nc.dram_tensor("dbg_qT", [128, S], bf16, kind="ExternalOutput")
        dbg_kvT = nc.dram_tensor("dbg_kvT", [128, S], bf16, kind="ExternalOutput")
        dbg_vaug = nc.dram_tensor("dbg_vaug", [128, NKT, 128], bf16, kind="ExternalOutput")
        dbg_ctxT = nc.dram_tensor("dbg_ctxT", [64, 2, S], bf16, kind="ExternalOutput")

    with ExitStack() as ctx:
        tc = ctx.enter_context(tile.TileContext(nc))
        singles = ctx.enter_context(tc.tile_pool(name="singles", bufs=1))
        pt_pool = ctx.enter_context(tc.tile_pool(name="pt", bufs=4))
        small = ctx.enter_context(tc.tile_pool(name="small", bufs=3))
        ostage = ctx.enter_context(tc.tile_pool(name="ostage", bufs=3))
        psum = ctx.enter_context(tc.tile_pool(name="psum", bufs=2, space="PSUM"))

        # ---- constant / persistent SBUF tensors ----
        xT_sb = singles.tile([128, 8, S], bf16, tag="xT")
        wqT_sb = singles.tile([128, 8, 128], bf16, tag="wqT")
        wkvT_sb = singles.tile([128, 8, 128], bf16, tag="wkvT")
        woT_sb = singles.tile([64, 2, DIM], bf16, tag="woT")
        mask_sb = singles.tile([128, 128], bf16, tag="mask")
        qT_sb = singles.tile([128, S], bf16, tag="qT")
        qT1_sb = singles.tile([64, S], bf16, tag="qT1")
        kvT_sb = singles.tile([128, S], bf16, tag="kvT")
        vaug_sb = singles.tile([128, NKT, 128], bf16, tag="vaug")
        ctxT_sb = singles.tile([64, 2, S], bf16, tag="ctxT")

        for c in range(8):
            nc.sync.dma_start(
                out=xT_sb[:, c, :],
                in_=xT_d[:].rearrange("(c p) s -> c p s", p=128)[c],
            )
        nc.sync.dma_start(
            out=wqT_sb, in_=wqT_d[:].rearrange("(c p) m -> p c m", p=128)
        )
        nc.sync.dma_start(
            out=wkvT_sb, in_=wkvT_d[:].rearrange("(c p) m -> p c m", p=128)
        )
        nc.sync.dma_start(
            out=woT_sb, in_=woT_d[:].rearrange("(h p) e -> p h e", p=64)
        )
        nc.sync.dma_start(out=mask_sb, in_=mask_d[:])

        # ---- projections: qT and kvT ----
        for dst, w_sb in ((qT_sb, wqT_sb), (kvT_sb, wkvT_sb)):
            for n in range(NQC):
                ps = psum.tile([128, GROUP, QC], fp32, tag="ps_s")
                for c in range(8):
                    nc.tensor.matmul(
                        ps[:, 0, :],
                        w_sb[:, c, :],
                        xT_sb[:, c, n * QC:(n + 1) * QC],
                        start=(c == 0),
                        stop=(c == 7),
                    )
                nc.vector.tensor_copy(dst[:, n * QC:(n + 1) * QC], ps[:, 0, :])

        # head-1 q rows shifted to base partition 0 (matmul operand rule)
        for n in range(NQC):
            nc.sync.dma_start(
                out=qT1_sb[:, n * QC:(n + 1) * QC],
                in_=qT_sb[64:128, n * QC:(n + 1) * QC],
            )

        # ---- v (normal layout) + ones column ----
        nc.vector.memset(vaug_sb[:, :, 64:66], 1.0)
        for kt in range(NKT):
            nc.sync.dma_start_transpose(
                out=vaug_sb[:, kt, 0:64],
                in_=kvT_sb[64:128, kt * 128:(kt + 1) * 128],
            )

        # ---- attention + output, pipelined over q-chunks ----
        for qc in range(NQC):
            nkt = min(NKT, 4 * qc + 4)
            rec_writes = []
            for h in range(2):
                ctx_ps = psum.tile([65, QC], fp32, tag="ps_ctx")
                if h == 0:
                    qs = qT_sb[0:64, qc * QC:(qc + 1) * QC]
                else:
                    qs = qT1_sb[:, qc * QC:(qc + 1) * QC]
                for g0 in range(0, nkt, GROUP):
                    gn = min(GROUP, nkt - g0)
                    ps_s = psum.tile([128, GROUP, QC], fp32, tag="ps_s")
                    pt = pt_pool.tile([128, GROUP, QC], bf16, tag="pt")
                    for i in range(gn):
                        kt = g0 + i
                        nc.tensor.matmul(
                            ps_s[:, i, :],
                            kvT_sb[0:64, kt * 128:(kt + 1) * 128],
                            qs,
                            start=True,
                            stop=True,
                        )
                    nc.scalar.activation(
                        pt[:, 0:gn, :], ps_s[:, 0:gn, :], Exp, scale=0.125
                    )
                    for i in range(gn):
                        kt = g0 + i
                        r = kt - 4 * qc
                        if r >= 0:  # strip intersects the causal diagonal
                            if r >= 1:
                                nc.vector.memset(pt[:, i, 0:128 * r], 0.0)
                            nc.vector.tensor_mul(
                                pt[:, i, 128 * r:128 * (r + 1)],
                                pt[:, i, 128 * r:128 * (r + 1)],
                                mask_sb,
                            )
                    for i in range(gn):
                        kt = g0 + i
                        nc.tensor.matmul(
                            ctx_ps,
                            vaug_sb[:, kt, 0:65],
                            pt[:, i, :],
                            start=(kt == 0),
                            stop=(kt == nkt - 1),
                        )

                # ctx (raw) -> SBUF; denominators -> DRAM -> recip -> DRAM
                nc.vector.tensor_copy(
                    ctxT_sb[:, h, qc * QC:(qc + 1) * QC],
                    ctx_ps[0:64, :],
                )
                srow = small.tile([65, QC], fp32, tag="srow")
                nc.vector.tensor_copy(srow[64:65, :], ctx_ps[64:65, :])
                w1 = nc.sync.dma_start(
                    out=sums_d[h:h + 1, qc * QC:(qc + 1) * QC],
                    in_=srow[64:65, :],
                )
                cp = small.tile([128, 4], fp32, tag="cp")
                r1 = nc.sync.dma_start(
                    out=cp,
                    in_=sums_d[h, qc * QC:(qc + 1) * QC].rearrange(
                        "(c p) -> p c", p=128
                    ),
                )
                add_dep_helper(r1.ins, w1.ins, reason="sums dram RAW")
                rec = small.tile([128, 4], fp32, tag="rec")
                nc.vector.reciprocal(rec, cp)
                recb = small.tile([128, 4], bf16, tag="recb")
                nc.vector.tensor_copy(recb, rec)
                w2 = nc.sync.dma_start(
                    out=rec_d[h, qc * QC:(qc + 1) * QC].rearrange(
                        "(c p) -> p c", p=128
                    ),
                    in_=recb,
                )
                rb = small.tile([128, QC], bf16, tag="rb")
                src = rec_d[h, qc * QC:(qc + 1) * QC]
                import concourse.bass as _b
                r2 = nc.sync.dma_start(
                    out=rb,
                    in_=_b.AP(tensor=src.tensor, offset=src.offset,
                              ap=[[0, 128]] + list(src.ap)),
                )
                add_dep_helper(r2.ins, w2.ins, reason="recips dram RAW")
                rec_writes.append((rb, None))
                # normalize ctxT in place
                nc.vector.tensor_mul(
                    ctxT_sb[:, h, qc * QC:(qc + 1) * QC],
                    ctxT_sb[:, h, qc * QC:(qc + 1) * QC],
                    rb[0:64, :],
                )

            # ---- output projection for this q-chunk's 4 row blocks ----
            for j, rc in enumerate(range(4 * qc, 4 * qc + 4)):
                ps_o = psum.tile([128, GROUP, QC], fp32, tag="ps_s")
                for e in range(2):
                    for h in range(2):
                        nc.tensor.matmul(
                            ps_o[:, e, :],
                            ctxT_sb[:, h, rc * 128:(rc + 1) * 128],
                            woT_sb[:, h, e * 512:(e + 1) * 512],
                            start=(h == 0),
                            stop=(h == 1),
                        )
                ot = ostage.tile([128, DIM], fp32, tag="ot")
                if j % 2 == 0:
                    nc.vector.tensor_copy(ot[:, 0:512], ps_o[:, 0, :])
                    nc.vector.tensor_copy(ot[:, 512:1024], ps_o[:, 1, :])
                else:
                    nc.scalar.copy(ot[:, 0:512], ps_o[:, 0, :])
                    nc.scalar.copy(ot[:, 512:1024], ps_o[:, 1, :])
                nc.sync.dma_start(
                    out=out_d[rc * 128:(rc + 1) * 128, :], in_=ot
                )

        if debug:
            nc.sync.dma_start(out=dbg_qT[:], in_=qT_sb)
            nc.sync.dma_start(out=dbg_kvT[:], in_=kvT_sb)
            nc.sync.dma_start(out=dbg_vaug[:], in_=vaug_sb)
            nc.sync.dma_start(out=dbg_ctxT[:], in_=ctxT_sb)

    nc.compile()
    return nc


def _get_nc():
    if "nc" not in _CACHE:
        _CACHE["nc"] = _build_nc()
    return _CACHE["nc"]


def _prep_inputs(x, wq, wk, wv, wo):
    GS = NH // NKV
    x2 = np.asarray(x, np.float32).reshape(S, DIN)
    xT = np.ascontiguousarray(x2.T).astype(BF16)
    tri = (np.arange(128)[None, :] >= np.arange(128)[:, None]).astype(BF16)
    in_maps = []
    for c in range(NCORES):
        h0 = 2 * c
        g = h0 // GS
        wq_c = np.asarray(wq, np.float32)[h0 * HD:(h0 + 2) * HD, :]
        wkv_c = np.concatenate(
            [
                np.asarray(wk, np.float32)[g * HD:(g + 1) * HD, :],
                np.asarray(wv, np.float32)[g * HD:(g + 1) * HD, :],
            ],
            axis=0,
        )
        woT_c = np.asarray(wo, np.float32)[:, h0 * HD:(h0 + 2) * HD].T
        in_maps.append(
            {
                "xT": xT,
                "wqT": np.ascontiguousarray(wq_c.T).astype(BF16),
                "wkvT": np.ascontiguousarray(wkv_c.T).astype(BF16),
                "woT": np.ascontiguousarray(woT_c).astype(BF16),
                "trimask": tri,
            }
        )
    return in_maps


def _run(in_maps, trace=False):
    import sys
    if "/opt/trn_rl_repo" not in sys.path:
        sys.path.insert(0, "/opt/trn_rl_repo")
    from concourse.bass_utils import run_bass_kernel_spmd

    nc = _get_nc()
    res = run_bass_kernel_spmd(nc, in_maps, list(range(NCORES)), trace=trace)
    return res


def kernel(x, wq, wk, wv, wo):
    in_maps = _prep_inputs(x, wq, wk, wv, wo)
    res = _run(in_maps)
    parts = np.stack([np.asarray(r["out"], np.float32) for r in res.results])
    out = parts.sum(axis=0, dtype=np.float64).astype(np.float32)
    return out.reshape(1, S, DIM)



# revision 5
# speedup vs baseline: 2.1099x; 2.1099x over previous
"""GQA (16 q-heads / 4 KV groups, S=4096, D=1024, causal) on 8 TRN2 NeuronCores.

Sharding: tensor-parallel over query heads - 2 q-heads + their KV group per
core. wq/wk/wv column-sharded, wo row-sharded; the 8 partial outputs are
summed on the host (no device collectives needed).

Per-core program (all matmuls bf16, f32 PSUM accumulation):
  qT  = (wq_c @ x^T)          [128, 4096]  rows 0-63 head0, 64-127 head1
  kv  = (wkv_c @ x^T)         [128, 4096]  rows 0-63 kT, 64-127 vT
  kds = kT duplicated on partitions 0-63 AND 64-127 so both heads' score
        matmuls run CONCURRENTLY in the PE array (row-group tiling: K=64
        tiles at tile_position (0,0) and (64,0)).
  v   -> vaug [128, kt, 65] via DMA transpose (col 64 = ones for the
        softmax denominator row).
  per q-chunk (512), per 2-strip group: packed scores -> exp (ScalarE,
        1 in 3 groups use a DVE int-trick exp for head1) -> causal mask
        (GpSimd) -> ctx accumulate [65, 512] per head.
  softmax normalization fully on-chip: denominators (ctx row 64) ->
        reciprocal (DVE) -> broadcast across partitions with a K=1
        outer-product matmul against a ones column -> ctxT scaled on evict.
  out rows = ctxT_chunk^T @ woT in single K=128 matmuls (heads pre-summed
        by layout); out DMA on the GpSimd SWDGE queue.
Softmax uses no max-subtraction: s/8 ~ N(0,1) -> exp safe in f32/bf16.
Tail work (normalize + out-proj) of chunk qc is interleaved into chunk
qc+1's score groups so PE never stalls on the DVE normalize chain.
"""

import numpy as np
import ml_dtypes

BF16 = ml_dtypes.bfloat16

S = 4096
DIN = 1024
DIM = 1024
NH, NKV, HD = 16, 4, 64
NCORES = 8
QC = 512          # q chunk width
NQC = S // QC     # 8
NKT = S // 128    # 32 k strips
GROUP = 2         # k strips per PSUM score tile / exp instruction
N_WARM = 180      # PE warm-up dummies issued while xT streams in
K_EXP = float(np.log2(np.e) * 16.0)   # bf16 int-exp scale: s -> (s/8*log2e)*2^7
B_EXP = 16250.5                       # 127<<7 minus mantissa correction

_CACHE = {}


def _build_nc(debug=False):
    import concourse.bass as bass
    import concourse.mybir as mybir
    import concourse.tile as tile
    from concourse import bacc
    from contextlib import ExitStack

    fp32 = mybir.dt.float32
    bf16 = mybir.dt.bfloat16
    i16 = mybir.dt.int16
    Exp = mybir.ActivationFunctionType.Exp
    MULT = mybir.AluOpType.mult
    ADD = mybir.AluOpType.add

    nc = bacc.Bacc()
    xT_d = nc.dram_tensor("xT", [DIN, S], bf16, kind="ExternalInput")
    wqT_d = nc.dram_tensor("wqT", [DIN, 128], bf16, kind="ExternalInput")
    wkvT_d = nc.dram_tensor("wkvT", [DIN, 128], bf16, kind="ExternalInput")
    woT_d = nc.dram_tensor("woT", [128, DIM], bf16, kind="ExternalInput")
    mask_d = nc.dram_tensor("trimask", [128, 128], bf16, kind="ExternalInput")
    out_d = nc.dram_tensor("out", [S, DIM], fp32, kind="ExternalOutput")
    if debug:
        dbg_qT = nc.dram_tensor("dbg_qT", [128, S], bf16, kind="ExternalOutput")
        dbg_kds = nc.dram_tensor("dbg_kds", [128, S], bf16, kind="ExternalOutput")
        dbg_vaug = nc.dram_tensor("dbg_vaug", [128, NKT, 128], bf16, kind="ExternalOutput")
        dbg_ctxT = nc.dram_tensor("dbg_ctxT", [128, S], bf16, kind="ExternalOutput")

    with ExitStack() as ctx:
        tc = ctx.enter_context(tile.TileContext(nc))
        singles = ctx.enter_context(tc.tile_pool(name="singles", bufs=1))
        ptp = ctx.enter_context(tc.tile_pool(name="pt", bufs=4))
        small = ctx.enter_context(tc.tile_pool(name="small", bufs=2))
        ost = ctx.enter_context(tc.tile_pool(name="ostage", bufs=3))
        psum = ctx.enter_context(tc.tile_pool(name="psum", bufs=1, space="PSUM"))

        # ---- persistent SBUF tensors ----
        xT_sb = singles.tile([128, 8, S], bf16, tag="xT")
        wqT_sb = singles.tile([128, 8, 128], bf16, tag="wqT")
        wkvT_sb = singles.tile([128, 8, 128], bf16, tag="wkvT")
        woT_sb = singles.tile([128, DIM], bf16, tag="woT")
        mask_sb = singles.tile([128, 128], bf16, tag="mask")
        ones_sb = singles.tile([128, 128], bf16, tag="ones")
        qT_sb = singles.tile([128, S], bf16, tag="qT")
        kds_sb = singles.tile([128, S], bf16, tag="kds")
        vt_sb = singles.tile([128, S], bf16, tag="vt")        # rows 64-127 used
        vaug_sb = singles.tile([128, NKT, 128], bf16, tag="vaug")
        ctxT_sb = singles.tile([128, S], bf16, tag="ctxT")

        # ---- loads (sync + scalar HWDGE queues) ----
        nc.sync.dma_start(
            out=wqT_sb, in_=wqT_d[:].rearrange("(c p) m -> p c m", p=128)
        )
        nc.sync.dma_start(
            out=wkvT_sb, in_=wkvT_d[:].rearrange("(c p) m -> p c m", p=128)
        )
        nc.sync.dma_start(out=woT_sb, in_=woT_d[:])
        nc.sync.dma_start(out=mask_sb, in_=mask_d[:])
        for c in range(8):
            eng = nc.sync if c < 4 else nc.scalar
            eng.dma_start(
                out=xT_sb[:, c, :],
                in_=xT_d[:].rearrange("(c p) s -> c p s", p=128)[c],
            )
        nc.vector.memset(ones_sb, 1.0)
        nc.vector.memset(vaug_sb[:, :, 64:66], 1.0)

        # ---- PE warm-up: trip the HAM to K=8/8 while xT streams in ----
        warm = psum.tile([128, GROUP, QC], fp32, tag="po", bufs=1)
        for _ in range(N_WARM):
            nc.tensor.matmul(warm[:, 0, 0:128], ones_sb, ones_sb,
                             start=True, stop=True)

        # ---- projections: accumulate over the 8 DIN chunks per 512 tokens ----
        for which in range(2):
            w_sb = wqT_sb if which == 0 else wkvT_sb
            for n in range(NQC):
                sl = slice(n * QC, (n + 1) * QC)
                ps = psum.tile([128, GROUP, QC], fp32, tag="s", bufs=2)
                for c in range(8):
                    nc.tensor.matmul(
                        ps[:, 0, :], w_sb[:, c, :], xT_sb[:, c, sl],
                        start=(c == 0), stop=(c == 7),
                    )
                if which == 0:
                    nc.scalar.copy(qT_sb[:, sl], ps[:, 0, :])
                else:
                    nc.scalar.copy(kds_sb[0:64, sl], ps[0:64, 0, :])
                    nc.vector.tensor_copy(kds_sb[64:128, sl], ps[0:64, 0, :])
                    nc.vector.tensor_copy(vt_sb[64:128, sl], ps[64:128, 0, :])

        # v (normal [token, dim] layout) via DMA transpose, after projections
        for kt in range(NKT):
            nc.sync.dma_start_transpose(
                out=vaug_sb[:, kt, 0:64],
                in_=vt_sb[64:128, kt * 128:(kt + 1) * 128],
            )

        # ---- attention + interleaved tails ----
        pending_tail = []

        def drain_one():
            if pending_tail:
                pending_tail.pop(0)()

        def make_tail(qc, ctx0, ctx1):
            qsl = slice(qc * QC, (qc + 1) * QC)
            st = {}

            def t1():  # denominators -> reciprocal -> broadcast matmul
                rr0 = small.tile([128, QC], bf16, tag="rr", name="rr0")
                rr1 = small.tile([128, QC], bf16, tag="rr", name="rr1")
                with nc.allow_low_precision("bf16 softmax denominators"):
                    nc.vector.reciprocal(rr0[64:65, :], ctx0[64:65, :])
                    nc.vector.reciprocal(rr1[64:65, :], ctx1[64:65, :])
                po = psum.tile([128, GROUP, QC], fp32, tag="po", bufs=1,
                               name="po_rb")
                nc.tensor.matmul(po[:, 0, :], ones_sb[64:65, 0:128],
                                 rr0[64:65, :], start=True, stop=True)
                nc.tensor.matmul(po[:, 1, :], ones_sb[64:65, 0:128],
                                 rr1[64:65, :], start=True, stop=True)
                st["po_rb"] = po

            def t2():  # broadcast -> SBUF, scale ctx on eviction
                po = st.pop("po_rb")
                rb0 = small.tile([128, QC], bf16, tag="rb", name="rb0")
                rb1 = small.tile([128, QC], bf16, tag="rb", name="rb1")
                nc.vector.tensor_copy(rb0, po[:, 0, :])
                nc.vector.tensor_copy(rb1, po[:, 1, :])
                nc.vector.tensor_mul(ctxT_sb[0:64, qsl], ctx0[0:64, :],
                                     rb0[0:64, :])
                nc.vector.tensor_mul(ctxT_sb[64:128, qsl], ctx1[0:64, :],
                                     rb1[0:64, :])

            def outproj(j):
                def f():
                    rc = 4 * qc + j
                    rsl = slice(rc * 128, (rc + 1) * 128)
                    po = psum.tile([128, GROUP, QC], fp32, tag="po", bufs=1,
                                   name=f"po_o{j}")
                    for e in range(2):
                        nc.tensor.matmul(
                            po[:, e, :], ctxT_sb[:, rsl],
                            woT_sb[:, e * QC:(e + 1) * QC],
                            start=True, stop=True,
                        )
                    ot = ost.tile([128, DIM], fp32, tag="ot", name=f"ot{j}")
                    nc.vector.tensor_copy(ot[:, 0:QC], po[:, 0, :])
                    nc.vector.tensor_copy(ot[:, QC:DIM], po[:, 1, :])
                    nc.gpsimd.dma_start(out=out_d[rsl, :], in_=ot)
                return f

            return [t1, t2, outproj(0), outproj(1), outproj(2), outproj(3)]

        gctr = 0
        for qc in range(NQC):
            nkt = 4 * qc + 4
            qsl = slice(qc * QC, (qc + 1) * QC)
            ctx0 = psum.tile([65, QC], fp32, tag="ctx", bufs=2, name="ctx0")
            ctx1 = psum.tile([65, QC], fp32, tag="ctx", bufs=2, name="ctx1")
            pend = None

            def emit_ctx(item):
                g0, pA, pB = item
                for i in range(GROUP):
                    kt = g0 + i
                    st_ = (kt == 0)
                    sp_ = (kt == nkt - 1)
                    nc.tensor.matmul(ctx0, vaug_sb[:, kt, 0:65], pA[:, i, :],
                                     start=st_, stop=sp_)
                    nc.tensor.matmul(ctx1, vaug_sb[:, kt, 0:65], pB[:, i, :],
                                     start=st_, stop=sp_)

            for g0 in range(0, nkt, GROUP):
                psA = psum.tile([128, GROUP, QC], fp32, tag="s", bufs=2,
                                name="psA")
                psB = psum.tile([128, GROUP, QC], fp32, tag="s", bufs=2,
                                name="psB")
                ptA = ptp.tile([128, GROUP, QC], bf16, tag="ptA", name="ptA")
                ptB = ptp.tile([128, GROUP, QC], bf16, tag="ptB", name="ptB")
                for i in range(GROUP):
                    kt = g0 + i
                    ksl = slice(kt * 128, (kt + 1) * 128)
                    nc.tensor.matmul(psA[:, i, :], kds_sb[0:64, ksl],
                                     qT_sb[0:64, qsl], start=True, stop=True)
                    nc.tensor.matmul(psB[:, i, :], kds_sb[64:128, ksl],
                                     qT_sb[64:128, qsl], start=True, stop=True)
                nc.scalar.activation(ptA, psA, Exp, scale=0.125)
                if gctr % 3 == 2:
                    # head1 exp via DVE int-trick (bf16 bit pattern), ~2-3% p
                    # error, washed out by softmax normalization.
                    nc.vector.tensor_scalar(
                        out=ptB.bitcast(i16), in0=psB,
                        scalar1=K_EXP, scalar2=B_EXP, op0=MULT, op1=ADD,
                    )
                else:
                    nc.scalar.activation(ptB, psB, Exp, scale=0.125)
                for i in range(GROUP):
                    kt = g0 + i
                    r = kt - 4 * qc
                    if r >= 0:  # strip intersects the causal diagonal
                        for pt in (ptA, ptB):
                            if r >= 1:
                                nc.gpsimd.memset(pt[:, i, 0:128 * r], 0.0)
                            nc.gpsimd.tensor_mul(
                                pt[:, i, 128 * r:128 * (r + 1)],
                                pt[:, i, 128 * r:128 * (r + 1)],
                                mask_sb,
                            )
                drain_one()  # one tail stage of the previous q-chunk
                if pend is not None:
                    emit_ctx(pend)
                pend = (g0, ptA, ptB)
                gctr += 1
            emit_ctx(pend)

            for f in pending_tail:  # leftovers (early, short q-chunks)
                f()
            pending_tail = make_tail(qc, ctx0, ctx1)

        for f in pending_tail:
            f()

        if debug:
            nc.sync.dma_start(out=dbg_qT[:], in_=qT_sb)
            nc.sync.dma_start(out=dbg_kds[:], in_=kds_sb)
            nc.sync.dma_start(out=dbg_vaug[:], in_=vaug_sb)
            nc.sync.dma_start(out=dbg_ctxT[:], in_=ctxT_sb)

    nc.compile()
    return nc


def _get_nc():
    if "nc" not in _CACHE:
        _CACHE["nc"] = _build_nc()
    return _CACHE["nc"]


def _prep_inputs(x, wq, wk, wv, wo):
    GS = NH // NKV
    x2 = np.asarray(x, np.float32).reshape(S, DIN)
    xT = np.ascontiguousarray(x2.T).astype(BF16)
    tri = (np.arange(128)[None, :] >= np.arange(128)[:, None]).astype(BF16)
    in_maps = []
    for c in range(NCORES):
        h0 = 2 * c
        g = h0 // GS
        wq_c = np.asarray(wq, np.float32)[h0 * HD:(h0 + 2) * HD, :]
        wkv_c = np.concatenate(
            [
                np.asarray(wk, np.float32)[g * HD:(g + 1) * HD, :],
                np.asarray(wv, np.float32)[g * HD:(g + 1) * HD, :],
            ],
            axis=0,
        )
        woT_c = np.asarray(wo, np.float32)[:, h0 * HD:(h0 + 2) * HD].T
        in_maps.append(
            {
                "xT": xT,
                "wqT": np.ascontiguousarray(wq_c.T).astype(BF16),
                "wkvT": np.ascontiguousarray(wkv_c.T).astype(BF16),
                "woT": np.ascontiguousarray(woT_c).astype(BF16),
                "trimask": tri,
            }
        )
    return in_maps


def _run(in_maps, trace=False):
    import sys
    if "/opt/trn_rl_repo" not in sys.path:
        sys.path.insert(0, "/opt/trn_rl_repo")
    from concourse.bass_utils import run_bass_kernel_spmd

    nc = _get_nc()
    res = run_bass_kernel_spmd(nc, in_maps, list(range(NCORES)), trace=trace)
    return res


def kernel(x, wq, wk, wv, wo):
    in_maps = _prep_inputs(x, wq, wk, wv, wo)
    res = _run(in_maps)
    parts = np.stack([np.asarray(r["out"], np.float32) for r in res.results])
    out = parts.sum(axis=0, dtype=np.float64).astype(np.float32)
    return out.reshape(1, S, DIM)


# revision 13
# speedup vs baseline: 2.1161x; 1.0029x over previous
"""GQA (16 q-heads / 4 KV groups, S=4096, D=1024, causal) on 8 TRN2 NeuronCores.

Sharding: tensor-parallel over query heads - 2 q-heads + their KV group per
core. wq/wk/wv column-sharded, wo row-sharded; the 8 partial outputs are
summed on the host (no device collectives needed).

Per-core program (all matmuls bf16, f32 PSUM accumulation):
  qT  = (wq_c @ x^T)          [128, 4096]  rows 0-63 head0, 64-127 head1
  kv  = (wkv_c @ x^T)         [128, 4096]  rows 0-63 kT, 64-127 vT
  kds = kT duplicated on partitions 0-63 AND 64-127 so both heads' score
        matmuls run CONCURRENTLY in the PE array (row-group tiling: K=64
        tiles at tile_position (0,0) and (64,0)).
  v   -> vaug [128, kt, 65] via DMA transpose (col 64 = ones for the
        softmax denominator row).
  per q-chunk (512), per 2-strip group: packed scores -> exp (ScalarE,
        1 in 3 groups use a DVE int-trick exp for head1) -> causal mask
        (GpSimd) -> ctx accumulate [65, 512] per head.
  softmax normalization fully on-chip: denominators (ctx row 64) ->
        reciprocal (DVE) -> broadcast across partitions with a K=1
        outer-product matmul against a ones column -> ctxT scaled on evict.
  out rows = ctxT_chunk^T @ woT in single K=128 matmuls (heads pre-summed
        by layout); out DMA on the GpSimd SWDGE queue.
Softmax uses no max-subtraction: s/8 ~ N(0,1) -> exp safe in f32/bf16.
Tail work (normalize + out-proj) of chunk qc is interleaved into chunk
qc+1's score groups so PE never stalls on the DVE normalize chain.
"""

import numpy as np
import ml_dtypes

BF16 = ml_dtypes.bfloat16

S = 4096
DIN = 1024
DIM = 1024
NH, NKV, HD = 16, 4, 64
NCORES = 8
QC = 512          # q chunk width
NQC = S // QC     # 8
NKT = S // 128    # 32 k strips
GROUP = 2         # k strips per PSUM score tile / exp instruction
N_WARM = 260      # PE warm-up dummies issued while xT streams in
K_EXP = float(np.log2(np.e) * 16.0)   # bf16 int-exp scale: s -> (s/8*log2e)*2^7
B_EXP = 16250.5                       # 127<<7 minus mantissa correction

_CACHE = {}


def _build_nc(debug=False):
    import concourse.bass as bass
    import concourse.mybir as mybir
    import concourse.tile as tile
    from concourse import bacc
    from contextlib import ExitStack

    fp32 = mybir.dt.float32
    bf16 = mybir.dt.bfloat16
    i16 = mybir.dt.int16
    Exp = mybir.ActivationFunctionType.Exp
    Ln = mybir.ActivationFunctionType.Ln
    MULT = mybir.AluOpType.mult
    ADD = mybir.AluOpType.add

    nc = bacc.Bacc()
    xT_d = nc.dram_tensor("xT", [DIN, S], bf16, kind="ExternalInput")
    wqT_d = nc.dram_tensor("wqT", [DIN, 128], bf16, kind="ExternalInput")
    wkvT_d = nc.dram_tensor("wkvT", [DIN, 128], bf16, kind="ExternalInput")
    woT_d = nc.dram_tensor("woT", [128, DIM], bf16, kind="ExternalInput")
    mask_d = nc.dram_tensor("trimask", [128, 128], bf16, kind="ExternalInput")
    out_d = nc.dram_tensor("out", [S, DIM], fp32, kind="ExternalOutput")
    if debug:
        dbg_qT = nc.dram_tensor("dbg_qT", [128, S], bf16, kind="ExternalOutput")
        dbg_kds = nc.dram_tensor("dbg_kds", [128, S], bf16, kind="ExternalOutput")
        dbg_vaug = nc.dram_tensor("dbg_vaug", [128, NKT, 128], bf16, kind="ExternalOutput")
        dbg_ctxT = nc.dram_tensor("dbg_ctxT", [128, S], bf16, kind="ExternalOutput")

    with ExitStack() as ctx:
        tc = ctx.enter_context(tile.TileContext(nc))
        singles = ctx.enter_context(tc.tile_pool(name="singles", bufs=1))
        ptp = ctx.enter_context(tc.tile_pool(name="pt", bufs=4))
        small = ctx.enter_context(tc.tile_pool(name="small", bufs=2))
        ost = ctx.enter_context(tc.tile_pool(name="ostage", bufs=3))
        psum = ctx.enter_context(tc.tile_pool(name="psum", bufs=1, space="PSUM"))

        # ---- persistent SBUF tensors ----
        xT_sb = singles.tile([128, 8, S], bf16, tag="xT")
        wqT_sb = singles.tile([128, 8, 128], bf16, tag="wqT")
        wkvT_sb = singles.tile([128, 8, 128], bf16, tag="wkvT")
        woT_sb = singles.tile([128, DIM], bf16, tag="woT")
        mask_sb = singles.tile([128, 128], bf16, tag="mask")
        ones_sb = singles.tile([128, 128], bf16, tag="ones")
        ones32_sb = singles.tile([128, 128], fp32, tag="ones32")
        qT_sb = singles.tile([128, S], bf16, tag="qT")
        kds_sb = singles.tile([128, S], bf16, tag="kds")
        vt_sb = singles.tile([128, S], bf16, tag="vt")        # rows 64-127 used
        vaug_sb = singles.tile([128, NKT, 128], bf16, tag="vaug")
        ctxT_sb = singles.tile([128, S], bf16, tag="ctxT")

        # ---- loads (sync + scalar HWDGE queues) ----
        nc.sync.dma_start(
            out=wqT_sb, in_=wqT_d[:].rearrange("(c p) m -> p c m", p=128)
        )
        nc.sync.dma_start(
            out=wkvT_sb, in_=wkvT_d[:].rearrange("(c p) m -> p c m", p=128)
        )
        nc.sync.dma_start(out=woT_sb, in_=woT_d[:])
        nc.sync.dma_start(out=mask_sb, in_=mask_d[:])
        for c in range(8):
            eng = nc.sync if c < 4 else nc.scalar
            eng.dma_start(
                out=xT_sb[:, c, :],
                in_=xT_d[:].rearrange("(c p) s -> c p s", p=128)[c],
            )
        nc.vector.memset(ones_sb, 1.0)
        nc.vector.memset(ones32_sb, 1.0)
        nc.vector.memset(vaug_sb[:, :, 64:66], 1.0)

        # ---- PE warm-up: trip the HAM to K=8/8 while xT streams in ----
        warm = psum.tile([128, GROUP, QC], fp32, tag="po", bufs=1)
        for _ in range(N_WARM):
            nc.tensor.matmul(warm[:, 0, 0:128], ones_sb, ones_sb,
                             start=True, stop=True)

        # ---- projections: accumulate over the 8 DIN chunks per 512 tokens ----
        for which in range(2):
            w_sb = wqT_sb if which == 0 else wkvT_sb
            for n in range(NQC):
                sl = slice(n * QC, (n + 1) * QC)
                ps = psum.tile([128, GROUP, QC], fp32, tag="s", bufs=2)
                for c in range(8):
                    nc.tensor.matmul(
                        ps[:, 0, :], w_sb[:, c, :], xT_sb[:, c, sl],
                        start=(c == 0), stop=(c == 7),
                    )
                if which == 0:
                    nc.vector.tensor_copy(qT_sb[:, sl], ps[:, 0, :])
                else:
                    nc.vector.tensor_copy(kds_sb[0:64, sl], ps[0:64, 0, :])
                    nc.vector.tensor_copy(kds_sb[64:128, sl], ps[0:64, 0, :])
                    nc.vector.tensor_copy(vt_sb[64:128, sl], ps[64:128, 0, :])

        # v (normal [token, dim] layout) via DMA transpose, after projections
        for kt in range(NKT):
            nc.sync.dma_start_transpose(
                out=vaug_sb[:, kt, 0:64],
                in_=vt_sb[64:128, kt * 128:(kt + 1) * 128],
            )

        # ---- attention + interleaved tails ----
        pending_tail = []

        def drain_one():
            if pending_tail:
                pending_tail.pop(0)()

        def make_tail(qc, ctx0, ctx1):
            qsl = slice(qc * QC, (qc + 1) * QC)
            st = {}

            def t1():  # head0 normalize chain first (frees ctx0 bank ASAP)
                # 1/den as exp(-ln den): both in the natural_log_exp_and_others
                # ACT table set, so no table switching with the softmax exps.
                ld0 = small.tile([128, QC], fp32, tag="ld", name="ld0")
                ld1 = small.tile([128, QC], fp32, tag="ld", name="ld1")
                rr0 = small.tile([128, QC], bf16, tag="rr", name="rr0")
                rr1 = small.tile([128, QC], bf16, tag="rr", name="rr1")
                po = psum.tile([128, GROUP, QC], fp32, tag="po", bufs=1,
                               name="po_rb")
                nc.scalar.activation(ld0[64:65, :], ctx0[64:65, :], Ln)
                nc.scalar.activation(rr0[64:65, :], ld0[64:65, :], Exp,
                                     scale=-1.0)
                nc.tensor.matmul(po[:, 0, :], ones_sb[64:65, 0:128],
                                 rr0[64:65, :], start=True, stop=True)
                rb0 = small.tile([128, QC], bf16, tag="rb", name="rb0")
                nc.vector.tensor_copy(rb0, po[:, 0, :])
                nc.vector.tensor_mul(ctxT_sb[0:64, qsl], ctx0[0:64, :],
                                     rb0[0:64, :])
                nc.scalar.activation(ld1[64:65, :], ctx1[64:65, :], Ln)
                nc.scalar.activation(rr1[64:65, :], ld1[64:65, :], Exp,
                                     scale=-1.0)
                nc.tensor.matmul(po[:, 1, :], ones_sb[64:65, 0:128],
                                 rr1[64:65, :], start=True, stop=True)
                st["po_rb"] = po

            def t2():  # head1 normalize
                po = st.pop("po_rb")
                rb1 = small.tile([128, QC], bf16, tag="rb", name="rb1")
                nc.vector.tensor_copy(rb1, po[:, 1, :])
                nc.vector.tensor_mul(ctxT_sb[64:128, qsl], ctx1[0:64, :],
                                     rb1[0:64, :])

            def outproj(j):
                def f():
                    rc = 4 * qc + j
                    rsl = slice(rc * 128, (rc + 1) * 128)
                    po = psum.tile([128, GROUP, QC], fp32, tag="po", bufs=1,
                                   name=f"po_o{j}")
                    for e in range(2):
                        nc.tensor.matmul(
                            po[:, e, :], ctxT_sb[:, rsl],
                            woT_sb[:, e * QC:(e + 1) * QC],
                            start=True, stop=True,
                        )
                    ot = ost.tile([128, DIM], fp32, tag="ot", name=f"ot{j}")
                    nc.vector.tensor_copy(ot[:, 0:QC], po[:, 0, :])
                    nc.vector.tensor_copy(ot[:, QC:DIM], po[:, 1, :])
                    nc.gpsimd.dma_start(out=out_d[rsl, :], in_=ot)
                return f

            return [t1, t2, outproj(0), outproj(1), outproj(2), outproj(3)]

        gctr = 0
        for qc in range(NQC):
            nkt = 4 * qc + 4
            qsl = slice(qc * QC, (qc + 1) * QC)
            ctx0 = psum.tile([65, QC], fp32, tag="ctx", bufs=2, name="ctx0")
            ctx1 = psum.tile([65, QC], fp32, tag="ctx", bufs=2, name="ctx1")
            pend = None

            def emit_ctx(item):
                g0, pA, pB = item
                for i in range(GROUP):
                    kt = g0 + i
                    st_ = (kt == 0)
                    sp_ = (kt == nkt - 1)
                    nc.tensor.matmul(ctx0, vaug_sb[:, kt, 0:65], pA[:, i, :],
                                     start=st_, stop=sp_)
                    nc.tensor.matmul(ctx1, vaug_sb[:, kt, 0:65], pB[:, i, :],
                                     start=st_, stop=sp_)

            for g0 in range(0, nkt, GROUP):
                psA = psum.tile([128, GROUP, QC], fp32, tag="s", bufs=2,
                                name="psA")
                psB = psum.tile([128, GROUP, QC], fp32, tag="s", bufs=2,
                                name="psB")
                ptA = ptp.tile([128, GROUP, QC], bf16, tag="ptA", name="ptA")
                ptB = ptp.tile([128, GROUP, QC], bf16, tag="ptB", name="ptB")
                for i in range(GROUP):
                    kt = g0 + i
                    ksl = slice(kt * 128, (kt + 1) * 128)
                    nc.tensor.matmul(psA[:, i, :], kds_sb[0:64, ksl],
                                     qT_sb[0:64, qsl], start=True, stop=True)
                    nc.tensor.matmul(psB[:, i, :], kds_sb[64:128, ksl],
                                     qT_sb[64:128, qsl], start=True, stop=True)
                nc.scalar.activation(ptA, psA, Exp, scale=0.125)
                if gctr % 2 == 1:
                    # head1 exp via DVE int-trick (bf16 bit pattern), ~2-3% p
                    # error, washed out by softmax normalization.
                    nc.vector.tensor_scalar(
                        out=ptB.bitcast(i16), in0=psB,
                        scalar1=K_EXP, scalar2=B_EXP, op0=MULT, op1=ADD,
                    )
                else:
                    nc.scalar.activation(ptB, psB, Exp, scale=0.125)
                for i in range(GROUP):
                    kt = g0 + i
                    r = kt - 4 * qc
                    if r >= 0:  # strip intersects the causal diagonal
                        for pt in (ptA, ptB):
                            if r >= 1:
                                nc.gpsimd.memset(pt[:, i, 0:128 * r], 0.0)
                            nc.gpsimd.tensor_mul(
                                pt[:, i, 128 * r:128 * (r + 1)],
                                pt[:, i, 128 * r:128 * (r + 1)],
                                mask_sb,
                            )
                drain_one()  # one tail stage of the previous q-chunk
                if pend is not None:
                    emit_ctx(pend)
                pend = (g0, ptA, ptB)
                gctr += 1
            emit_ctx(pend)

            for f in pending_tail:  # leftovers (early, short q-chunks)
                f()
            pending_tail = make_tail(qc, ctx0, ctx1)

        for f in pending_tail:
            f()

        if debug:
            nc.sync.dma_start(out=dbg_qT[:], in_=qT_sb)
            nc.sync.dma_start(out=dbg_kds[:], in_=kds_sb)
            nc.sync.dma_start(out=dbg_vaug[:], in_=vaug_sb)
            nc.sync.dma_start(out=dbg_ctxT[:], in_=ctxT_sb)

    nc.compile()
    return nc


def _get_nc():
    if "nc" not in _CACHE:
        _CACHE["nc"] = _build_nc()
    return _CACHE["nc"]


def _prep_inputs(x, wq, wk, wv, wo):
    GS = NH // NKV
    x2 = np.asarray(x, np.float32).reshape(S, DIN)
    xT = np.ascontiguousarray(x2.T).astype(BF16)
    tri = (np.arange(128)[None, :] >= np.arange(128)[:, None]).astype(BF16)
    in_maps = []
    for c in range(NCORES):
        h0 = 2 * c
        g = h0 // GS
        wq_c = np.asarray(wq, np.float32)[h0 * HD:(h0 + 2) * HD, :]
        wkv_c = np.concatenate(
            [
                np.asarray(wk, np.float32)[g * HD:(g + 1) * HD, :],
                np.asarray(wv, np.float32)[g * HD:(g + 1) * HD, :],
            ],
            axis=0,
        )
        woT_c = np.asarray(wo, np.float32)[:, h0 * HD:(h0 + 2) * HD].T
        in_maps.append(
            {
                "xT": xT,
                "wqT": np.ascontiguousarray(wq_c.T).astype(BF16),
                "wkvT": np.ascontiguousarray(wkv_c.T).astype(BF16),
                "woT": np.ascontiguousarray(woT_c).astype(BF16),
                "trimask": tri,
            }
        )
    return in_maps


def _run(in_maps, trace=False):
    import sys
    if "/opt/trn_rl_repo" not in sys.path:
        sys.path.insert(0, "/opt/trn_rl_repo")
    from concourse.bass_utils import run_bass_kernel_spmd

    nc = _get_nc()
    res = run_bass_kernel_spmd(nc, in_maps, list(range(NCORES)), trace=trace)
    return res


def kernel(x, wq, wk, wv, wo):
    in_maps = _prep_inputs(x, wq, wk, wv, wo)
    res = _run(in_maps)
    parts = np.stack([np.asarray(r["out"], np.float32) for r in res.results])
    out = parts.sum(axis=0, dtype=np.float64).astype(np.float32)
    return out.reshape(1, S, DIM)


# revision 21
# speedup vs baseline: 2.1957x; 1.0376x over previous
"""GQA (16 q-heads / 4 KV groups, S=4096, D=1024, causal) on 8 TRN2 NeuronCores.

Sharding: tensor-parallel over query heads - 2 q-heads + their KV group per
core. wq/wk/wv column-sharded, wo row-sharded; the 8 partial outputs are
summed on the host (no device collectives needed).

Per-core program (all matmuls bf16, f32 PSUM accumulation):
  qT  = (wq_c @ x^T)          [128, 4096]  rows 0-63 head0, 64-127 head1
  kv  = (wkv_c @ x^T)         [128, 4096]  rows 0-63 kT, 64-127 vT
  kds = kT duplicated on partitions 0-63 AND 64-127 so both heads' score
        matmuls run CONCURRENTLY in the PE array (row-group tiling: K=64
        tiles at tile_position (0,0) and (64,0)).
  v   -> vaug [128, kt, 65] via DMA transpose (col 64 = ones for the
        softmax denominator row).
  per q-chunk (512), per 2-strip group: packed scores -> exp (ScalarE,
        1 in 3 groups use a DVE int-trick exp for head1) -> causal mask
        (GpSimd) -> ctx accumulate [65, 512] per head.
  softmax normalization fully on-chip: denominators (ctx row 64) ->
        reciprocal (DVE) -> broadcast across partitions with a K=1
        outer-product matmul against a ones column -> ctxT scaled on evict.
  out rows = ctxT_chunk^T @ woT in single K=128 matmuls (heads pre-summed
        by layout); out DMA on the GpSimd SWDGE queue.
Softmax uses no max-subtraction: s/8 ~ N(0,1) -> exp safe in f32/bf16.
Tail work (normalize + out-proj) of chunk qc is interleaved into chunk
qc+1's score groups so PE never stalls on the DVE normalize chain.
"""

import numpy as np
import ml_dtypes

BF16 = ml_dtypes.bfloat16

S = 4096
DIN = 1024
DIM = 1024
NH, NKV, HD = 16, 4, 64
NCORES = 8
QC = 512          # q chunk width
NQC = S // QC     # 8
NKT = S // 128    # 32 k strips
GROUP = 2         # k strips per PSUM score tile / exp instruction
N_WARM = 130      # PE warm-up matmuls (N=512) issued while xT streams in
K_EXP = float(np.log2(np.e) * 16.0)   # bf16 int-exp scale: s -> (s/8*log2e)*2^7
B_EXP = 16250.5                       # 127<<7 minus mantissa correction
C_RECIP = float(0x7EF311C3)           # int-Newton reciprocal seed magic

_CACHE = {}


def _build_nc(debug=False):
    import concourse.bass as bass
    import concourse.mybir as mybir
    import concourse.tile as tile
    from concourse import bacc
    from contextlib import ExitStack

    fp32 = mybir.dt.float32
    bf16 = mybir.dt.bfloat16
    i16 = mybir.dt.int16
    i32 = mybir.dt.int32
    Exp = mybir.ActivationFunctionType.Exp
    MULT = mybir.AluOpType.mult
    ADD = mybir.AluOpType.add
    SUB = mybir.AluOpType.subtract

    nc = bacc.Bacc()
    xT_d = nc.dram_tensor("xT", [DIN, S], bf16, kind="ExternalInput")
    wqT_d = nc.dram_tensor("wqT", [DIN, 128], bf16, kind="ExternalInput")
    wkvT_d = nc.dram_tensor("wkvT", [DIN, 128], bf16, kind="ExternalInput")
    woT_d = nc.dram_tensor("woT", [128, DIM], bf16, kind="ExternalInput")
    mask_d = nc.dram_tensor("trimask", [128, 128], bf16, kind="ExternalInput")
    out_d = nc.dram_tensor("out", [S, DIM], fp32, kind="ExternalOutput")
    if debug:
        dbg_qT = nc.dram_tensor("dbg_qT", [128, S], bf16, kind="ExternalOutput")
        dbg_kds = nc.dram_tensor("dbg_kds", [128, S], bf16, kind="ExternalOutput")
        dbg_vaug = nc.dram_tensor("dbg_vaug", [128, NKT, 128], bf16, kind="ExternalOutput")
        dbg_ctxT = nc.dram_tensor("dbg_ctxT", [128, S], bf16, kind="ExternalOutput")

    with ExitStack() as ctx:
        tc = ctx.enter_context(tile.TileContext(nc))
        singles = ctx.enter_context(tc.tile_pool(name="singles", bufs=1))
        ptp = ctx.enter_context(tc.tile_pool(name="pt", bufs=4))
        small = ctx.enter_context(tc.tile_pool(name="small", bufs=2))
        ost = ctx.enter_context(tc.tile_pool(name="ostage", bufs=3))
        psum = ctx.enter_context(tc.tile_pool(name="psum", bufs=1, space="PSUM"))

        # ---- persistent SBUF tensors ----
        xT_sb = singles.tile([128, 8, S], bf16, tag="xT")
        wqT_sb = singles.tile([128, 8, 128], bf16, tag="wqT")
        wkvT_sb = singles.tile([128, 8, 128], bf16, tag="wkvT")
        woT_sb = singles.tile([128, DIM], bf16, tag="woT")
        mask_sb = singles.tile([128, 128], bf16, tag="mask")
        ones_sb = singles.tile([128, 128], bf16, tag="ones")
        mones_sb = singles.tile([128, 128], bf16, tag="mones")
        qT_sb = singles.tile([128, S], bf16, tag="qT")
        kds_sb = singles.tile([128, S], bf16, tag="kds")
        vt_sb = singles.tile([128, S], bf16, tag="vt")        # rows 64-127 used
        vaug_sb = singles.tile([128, NKT, 128], bf16, tag="vaug")
        ctxT_sb = singles.tile([128, S], bf16, tag="ctxT")

        # ---- loads (sync + scalar HWDGE queues) ----
        nc.sync.dma_start(
            out=wqT_sb, in_=wqT_d[:].rearrange("(c p) m -> p c m", p=128)
        )
        nc.sync.dma_start(
            out=wkvT_sb, in_=wkvT_d[:].rearrange("(c p) m -> p c m", p=128)
        )
        nc.sync.dma_start(out=woT_sb, in_=woT_d[:])
        nc.sync.dma_start(out=mask_sb, in_=mask_d[:])
        for c in range(8):
            eng = nc.sync if c < 4 else nc.scalar
            eng.dma_start(
                out=xT_sb[:, c, :],
                in_=xT_d[:].rearrange("(c p) s -> c p s", p=128)[c],
            )
        nc.vector.memset(ones_sb, 1.0)
        nc.vector.memset(mones_sb, -1.0)
        nc.vector.memset(vaug_sb[:, :, 64:66], 1.0)

        # ---- PE warm-up: keep the HAM at K=8/8 while xT streams in ----
        warm = psum.tile([128, GROUP, QC], fp32, tag="po", bufs=1)
        for _ in range(N_WARM):
            nc.tensor.matmul(warm[:, 0, :], wqT_sb[:, 0, :],
                             wkvT_sb[:, 0:4, :], start=True, stop=True)

        # ---- projections: accumulate over the 8 DIN chunks per 512 tokens ----
        for which in range(2):
            w_sb = wqT_sb if which == 0 else wkvT_sb
            for n in range(NQC):
                sl = slice(n * QC, (n + 1) * QC)
                ps = psum.tile([128, GROUP, QC], fp32, tag="s", bufs=2)
                for c in range(8):
                    nc.tensor.matmul(
                        ps[:, 0, :], w_sb[:, c, :], xT_sb[:, c, sl],
                        start=(c == 0), stop=(c == 7),
                    )
                if which == 0:
                    nc.vector.tensor_copy(qT_sb[:, sl], ps[:, 0, :])
                else:
                    nc.vector.tensor_copy(kds_sb[0:64, sl], ps[0:64, 0, :])
                    nc.vector.tensor_copy(kds_sb[64:128, sl], ps[0:64, 0, :])
                    nc.vector.tensor_copy(vt_sb[64:128, sl], ps[64:128, 0, :])

        # v (normal [token, dim] layout) via DMA transpose, after projections
        for kt in range(NKT):
            nc.sync.dma_start_transpose(
                out=vaug_sb[:, kt, 0:64],
                in_=vt_sb[64:128, kt * 128:(kt + 1) * 128],
            )

        # ---- attention + interleaved tails ----
        pending_tail = []

        def drain_one():
            if pending_tail:
                pending_tail.pop(0)()

        def make_tail(qc, ctx0, ctx1):
            qsl = slice(qc * QC, (qc + 1) * QC)
            st = {}

            def headchain(ctx_h, h, po):
                # raw ctx -> ctxT (frees the PSUM bank without waiting on the
                # reciprocal), then 1/den via integer-seed + one Newton step:
                #   r0 = bitcast(C - bits(den));  rr = (den*r0 - 2)*r0 = -1/den
                # broadcast with a minus-ones column restores the sign.
                nc.vector.tensor_copy(ctxT_sb[64 * h:64 * (h + 1), qsl],
                                      ctx_h[0:64, :])
                r0 = small.tile([128, QC], fp32, tag="r0", name=f"r0_{h}")
                nc.vector.tensor_scalar(
                    out=r0[64:65, :].bitcast(i32),
                    in0=ctx_h[64:65, :].bitcast(i32),
                    scalar1=-1.0, scalar2=C_RECIP, op0=MULT, op1=ADD,
                )
                tnw = small.tile([128, QC], fp32, tag="tnw", name=f"tnw{h}")
                nc.vector.tensor_mul(tnw[64:65, :], ctx_h[64:65, :],
                                     r0[64:65, :])
                rr = small.tile([128, QC], bf16, tag="rr", name=f"rr{h}")
                nc.vector.scalar_tensor_tensor(
                    out=rr[64:65, :], in0=tnw[64:65, :], scalar=2.0,
                    in1=r0[64:65, :], op0=SUB, op1=MULT,
                )
                nc.tensor.matmul(po[:, h, :], mones_sb[64:65, 0:128],
                                 rr[64:65, :], start=True, stop=True)

            def t1():
                po = psum.tile([128, GROUP, QC], fp32, tag="po", bufs=1,
                               name="po_rb")
                headchain(ctx0, 0, po)
                st["po_rb"] = po

            def t2():
                headchain(ctx1, 1, st["po_rb"])

            def t3():  # broadcasted recips -> normalize ctxT in place (bf16)
                po = st.pop("po_rb")
                rb0 = small.tile([128, QC], bf16, tag="rb", name="rb0")
                rb1 = small.tile([128, QC], bf16, tag="rb", name="rb1")
                nc.vector.tensor_copy(rb0, po[:, 0, :])
                nc.vector.tensor_mul(ctxT_sb[0:64, qsl],
                                     ctxT_sb[0:64, qsl], rb0[0:64, :])
                nc.vector.tensor_copy(rb1, po[:, 1, :])
                nc.vector.tensor_mul(ctxT_sb[64:128, qsl],
                                     ctxT_sb[64:128, qsl], rb1[64:128, :])

            def outproj(j):
                def f():
                    rc = 4 * qc + j
                    rsl = slice(rc * 128, (rc + 1) * 128)
                    po = psum.tile([128, GROUP, QC], fp32, tag="po", bufs=1,
                                   name=f"po_o{j}")
                    for e in range(2):
                        nc.tensor.matmul(
                            po[:, e, :], ctxT_sb[:, rsl],
                            woT_sb[:, e * QC:(e + 1) * QC],
                            start=True, stop=True,
                        )
                    ot = ost.tile([128, DIM], fp32, tag="ot", name=f"ot{j}")
                    nc.vector.tensor_copy(ot[:, 0:QC], po[:, 0, :])
                    nc.scalar.copy(ot[:, QC:DIM], po[:, 1, :])
                    nc.gpsimd.dma_start(out=out_d[rsl, :], in_=ot)
                return f

            return [t1, t2, t3, outproj(0), outproj(1), outproj(2), outproj(3)]

        gctr = 0
        for qc in range(NQC):
            nkt = 4 * qc + 4
            qsl = slice(qc * QC, (qc + 1) * QC)
            ctx0 = psum.tile([65, QC], fp32, tag="ctx", bufs=2, name="ctx0")
            ctx1 = psum.tile([65, QC], fp32, tag="ctx", bufs=2, name="ctx1")
            pend = None

            def emit_ctx(item):
                g0, pA, pB = item
                for i in range(GROUP):
                    kt = g0 + i
                    st_ = (kt == 0)
                    sp_ = (kt == nkt - 1)
                    nc.tensor.matmul(ctx0, vaug_sb[:, kt, 0:65], pA[:, i, :],
                                     start=st_, stop=sp_)
                    nc.tensor.matmul(ctx1, vaug_sb[:, kt, 0:65], pB[:, i, :],
                                     start=st_, stop=sp_)

            for g0 in range(0, nkt, GROUP):
                psA = psum.tile([128, GROUP, QC], fp32, tag="s", bufs=2,
                                name="psA")
                psB = psum.tile([128, GROUP, QC], fp32, tag="s", bufs=2,
                                name="psB")
                ptA = ptp.tile([128, GROUP, QC], bf16, tag="ptA", name="ptA")
                ptB = ptp.tile([128, GROUP, QC], bf16, tag="ptB", name="ptB")
                for i in range(GROUP):
                    kt = g0 + i
                    ksl = slice(kt * 128, (kt + 1) * 128)
                    nc.tensor.matmul(psA[:, i, :], kds_sb[0:64, ksl],
                                     qT_sb[0:64, qsl], start=True, stop=True)
                    nc.tensor.matmul(psB[:, i, :], kds_sb[64:128, ksl],
                                     qT_sb[64:128, qsl], start=True, stop=True)
                nc.scalar.activation(ptA, psA, Exp, scale=0.125)
                if gctr % 3 == 2:
                    # head1 exp via DVE int-trick (bf16 bit pattern), ~2-3% p
                    # error, washed out by softmax normalization.
                    nc.vector.tensor_scalar(
                        out=ptB.bitcast(i16), in0=psB,
                        scalar1=K_EXP, scalar2=B_EXP, op0=MULT, op1=ADD,
                    )
                else:
                    nc.scalar.activation(ptB, psB, Exp, scale=0.125)
                for i in range(GROUP):
                    kt = g0 + i
                    r = kt - 4 * qc
                    if r >= 0:  # strip intersects the causal diagonal
                        for pt in (ptA, ptB):
                            if r >= 1:
                                nc.gpsimd.memset(pt[:, i, 0:128 * r], 0.0)
                            nc.gpsimd.tensor_mul(
                                pt[:, i, 128 * r:128 * (r + 1)],
                                pt[:, i, 128 * r:128 * (r + 1)],
                                mask_sb,
                            )
                drain_one()  # one tail stage of the previous q-chunk
                if pend is not None:
                    emit_ctx(pend)
                pend = (g0, ptA, ptB)
                gctr += 1
            emit_ctx(pend)

            for f in pending_tail:  # leftovers (early, short q-chunks)
                f()
            pending_tail = make_tail(qc, ctx0, ctx1)

        for f in pending_tail:
            f()

        if debug:
            nc.sync.dma_start(out=dbg_qT[:], in_=qT_sb)
            nc.sync.dma_start(out=dbg_kds[:], in_=kds_sb)
            nc.sync.dma_start(out=dbg_vaug[:], in_=vaug_sb)
            nc.sync.dma_start(out=dbg_ctxT[:], in_=ctxT_sb)

    nc.compile()
    return nc


def _get_nc():
    if "nc" not in _CACHE:
        _CACHE["nc"] = _build_nc()
    return _CACHE["nc"]


def _prep_inputs(x, wq, wk, wv, wo):
    GS = NH // NKV
    x2 = np.asarray(x, np.float32).reshape(S, DIN)
    xT = np.ascontiguousarray(x2.T).astype(BF16)
    tri = (np.arange(128)[None, :] >= np.arange(128)[:, None]).astype(BF16)
    in_maps = []
    for c in range(NCORES):
        h0 = 2 * c
        g = h0 // GS
        wq_c = np.asarray(wq, np.float32)[h0 * HD:(h0 + 2) * HD, :]
        wkv_c = np.concatenate(
            [
                np.asarray(wk, np.float32)[g * HD:(g + 1) * HD, :],
                np.asarray(wv, np.float32)[g * HD:(g + 1) * HD, :],
            ],
            axis=0,
        )
        woT_c = np.asarray(wo, np.float32)[:, h0 * HD:(h0 + 2) * HD].T
        in_maps.append(
            {
                "xT": xT,
                "wqT": np.ascontiguousarray(wq_c.T).astype(BF16),
                "wkvT": np.ascontiguousarray(wkv_c.T).astype(BF16),
                "woT": np.ascontiguousarray(woT_c).astype(BF16),
                "trimask": tri,
            }
        )
    return in_maps


def _run(in_maps, trace=False):
    import sys
    if "/opt/trn_rl_repo" not in sys.path:
        sys.path.insert(0, "/opt/trn_rl_repo")
    from concourse.bass_utils import run_bass_kernel_spmd

    nc = _get_nc()
    res = run_bass_kernel_spmd(nc, in_maps, list(range(NCORES)), trace=trace)
    return res


def kernel(x, wq, wk, wv, wo):
    in_maps = _prep_inputs(x, wq, wk, wv, wo)
    res = _run(in_maps)
    parts = np.stack([np.asarray(r["out"], np.float32) for r in res.results])
    out = parts.sum(axis=0, dtype=np.float64).astype(np.float32)
    return out.reshape(1, S, DIM)


# revision 25
# speedup vs baseline: 2.2356x; 1.0182x over previous
"""GQA (16 q-heads / 4 KV groups, S=4096, D=1024, causal) on 8 TRN2 NeuronCores.

Sharding: tensor-parallel over query heads - 2 q-heads + their KV group per
core. wq/wk/wv column-sharded, wo row-sharded; the 8 partial outputs are
summed on the host (no device collectives needed).

Per-core program (all matmuls bf16, f32 PSUM accumulation):
  qT  = (wq_c @ x^T)          [128, 4096]  rows 0-63 head0, 64-127 head1
  kv  = (wkv_c @ x^T)         [128, 4096]  rows 0-63 kT, 64-127 vT
  kds = kT duplicated on partitions 0-63 AND 64-127 so both heads' score
        matmuls run CONCURRENTLY in the PE array (row-group tiling: K=64
        tiles at tile_position (0,0) and (64,0)).
  v   -> vaug [128, kt, 65] via DMA transpose (col 64 = ones for the
        softmax denominator row).
  per q-chunk (512), per 2-strip group: packed scores -> exp (ScalarE,
        1 in 3 groups use a DVE int-trick exp for head1) -> causal mask
        (GpSimd) -> ctx accumulate [65, 512] per head.
  softmax normalization fully on-chip: denominators (ctx row 64) ->
        reciprocal (DVE) -> broadcast across partitions with a K=1
        outer-product matmul against a ones column -> ctxT scaled on evict.
  out rows = ctxT_chunk^T @ woT in single K=128 matmuls (heads pre-summed
        by layout); out DMA on the GpSimd SWDGE queue.
Softmax uses no max-subtraction: s/8 ~ N(0,1) -> exp safe in f32/bf16.
Tail work (normalize + out-proj) of chunk qc is interleaved into chunk
qc+1's score groups so PE never stalls on the DVE normalize chain.
"""

import numpy as np
import ml_dtypes

BF16 = ml_dtypes.bfloat16

S = 4096
DIN = 1024
DIM = 1024
NH, NKV, HD = 16, 4, 64
NCORES = 8
QC = 512          # q chunk width
NQC = S // QC     # 8
NKT = S // 128    # 32 k strips
GROUP = 2         # k strips per PSUM score tile / exp instruction
N_WARM = 40       # PE warm-up matmuls until the first xT chunk lands
K_EXP = float(np.log2(np.e) * 16.0)   # bf16 int-exp scale: s -> (s/8*log2e)*2^7
B_EXP = 16250.5                       # 127<<7 minus mantissa correction
C_RECIP = float(0x7EF311C3)           # int-Newton reciprocal seed magic

_CACHE = {}


def _build_nc(debug=False):
    import concourse.bass as bass
    import concourse.mybir as mybir
    import concourse.tile as tile
    from concourse import bacc
    from contextlib import ExitStack

    fp32 = mybir.dt.float32
    bf16 = mybir.dt.bfloat16
    i16 = mybir.dt.int16
    i32 = mybir.dt.int32
    Exp = mybir.ActivationFunctionType.Exp
    MULT = mybir.AluOpType.mult
    ADD = mybir.AluOpType.add
    SUB = mybir.AluOpType.subtract

    nc = bacc.Bacc()
    xT_d = nc.dram_tensor("xT", [DIN, S], bf16, kind="ExternalInput")
    wqT_d = nc.dram_tensor("wqT", [DIN, 128], bf16, kind="ExternalInput")
    wkvT_d = nc.dram_tensor("wkvT", [DIN, 128], bf16, kind="ExternalInput")
    woT_d = nc.dram_tensor("woT", [128, DIM], bf16, kind="ExternalInput")
    mask_d = nc.dram_tensor("trimask", [128, 128], bf16, kind="ExternalInput")
    out_d = nc.dram_tensor("out", [S, DIM], fp32, kind="ExternalOutput")
    if debug:
        dbg_qT = nc.dram_tensor("dbg_qT", [128, S], bf16, kind="ExternalOutput")
        dbg_kds = nc.dram_tensor("dbg_kds", [128, S], bf16, kind="ExternalOutput")
        dbg_vaug = nc.dram_tensor("dbg_vaug", [128, NKT, 128], bf16, kind="ExternalOutput")
        dbg_ctxT = nc.dram_tensor("dbg_ctxT", [128, S], bf16, kind="ExternalOutput")

    with ExitStack() as ctx:
        tc = ctx.enter_context(tile.TileContext(nc))
        singles = ctx.enter_context(tc.tile_pool(name="singles", bufs=1))
        ptp = ctx.enter_context(tc.tile_pool(name="pt", bufs=4))
        small = ctx.enter_context(tc.tile_pool(name="small", bufs=2))
        ost = ctx.enter_context(tc.tile_pool(name="ostage", bufs=3))
        psum = ctx.enter_context(tc.tile_pool(name="psum", bufs=1, space="PSUM"))

        # ---- persistent SBUF tensors ----
        xT_sb = singles.tile([128, 8, S], bf16, tag="xT")
        wqT_sb = singles.tile([128, 8, 128], bf16, tag="wqT")
        wkvT_sb = singles.tile([128, 8, 128], bf16, tag="wkvT")
        woT_sb = singles.tile([128, DIM], bf16, tag="woT")
        mask_sb = singles.tile([128, 128], bf16, tag="mask")
        ones_sb = singles.tile([128, 128], bf16, tag="ones")
        mones_sb = singles.tile([128, 128], bf16, tag="mones")
        qT_sb = singles.tile([128, S], bf16, tag="qT")
        kds_sb = singles.tile([128, S], bf16, tag="kds")
        vt_sb = singles.tile([128, S], bf16, tag="vt")        # rows 64-127 used
        vaug_sb = singles.tile([128, NKT, 128], bf16, tag="vaug")
        ctxT_sb = singles.tile([128, S], bf16, tag="ctxT")

        # ---- loads: weights + 2 xT chunks on GpSimd SWDGE, 3+3 on sync/scalar
        nc.gpsimd.dma_start(
            out=wqT_sb, in_=wqT_d[:].rearrange("(c p) m -> p c m", p=128)
        )
        nc.gpsimd.dma_start(
            out=wkvT_sb, in_=wkvT_d[:].rearrange("(c p) m -> p c m", p=128)
        )
        nc.gpsimd.dma_start(out=woT_sb, in_=woT_d[:])
        nc.gpsimd.dma_start(out=mask_sb, in_=mask_d[:])
        for c in range(8):
            eng = (nc.sync, nc.scalar, nc.gpsimd)[c % 3] if c < 6 else (
                nc.sync if c == 6 else nc.scalar)
            eng.dma_start(
                out=xT_sb[:, c, :],
                in_=xT_d[:].rearrange("(c p) s -> c p s", p=128)[c],
            )
        nc.vector.memset(ones_sb, 1.0)
        nc.vector.memset(mones_sb, -1.0)
        nc.vector.memset(vaug_sb[:, :, 64:66], 1.0)

        # ---- PE warm-up until the first xT chunk lands ----
        warm = psum.tile([128, GROUP, QC], fp32, tag="po", bufs=1)
        for _ in range(N_WARM):
            nc.tensor.matmul(warm[:, 0, 0:128], ones_sb, ones_sb,
                             start=True, stop=True)

        # ---- projection helpers ----
        def evict_q(n, ps_slot):
            sl = slice(n * QC, (n + 1) * QC)
            nc.vector.tensor_copy(qT_sb[:, sl], ps_slot)

        def evict_kv(n, ps_slot):
            sl = slice(n * QC, (n + 1) * QC)
            nc.vector.tensor_copy(kds_sb[0:64, sl], ps_slot[0:64, :])
            nc.vector.tensor_copy(kds_sb[64:128, sl], ps_slot[0:64, :])
            nc.vector.tensor_copy(vt_sb[64:128, sl], ps_slot[64:128, :])
            for t in range(4):
                kt = 4 * n + t
                nc.sync.dma_start_transpose(
                    out=vaug_sb[:, kt, 0:64],
                    in_=vt_sb[64:128, kt * 128:(kt + 1) * 128],
                )

        # ---- c-outer projections for token chunks 0-1: each xT chunk is
        # consumed as it lands, so these finish right after the load ----
        sA = psum.tile([128, GROUP, QC], fp32, tag="s", bufs=2, name="psA")
        sB = psum.tile([128, GROUP, QC], fp32, tag="s", bufs=2, name="psB")
        plan = [(wqT_sb, sA, 0), (wkvT_sb, sB, 0), (wkvT_sb, sB, 1),
                (wqT_sb, sA, 1)]
        for c in range(8):
            for w_sb, tl, n in plan:
                nc.tensor.matmul(
                    tl[:, n, :], w_sb[:, c, :],
                    xT_sb[:, c, n * QC:(n + 1) * QC],
                    start=(c == 0), stop=(c == 7),
                )
        evict_kv(0, sB[:, 0, :])
        evict_q(0, sA[:, 0, :])
        evict_kv(1, sB[:, 1, :])
        evict_q(1, sA[:, 1, :])

        def proj_rest():
            # remaining token chunks, standard n-outer accumulation
            for which, w_sb in ((0, wqT_sb), (1, wkvT_sb)):
                for n in range(2, NQC):
                    sl = slice(n * QC, (n + 1) * QC)
                    ps = psum.tile([128, GROUP, QC], fp32, tag="s", bufs=2)
                    for c in range(8):
                        nc.tensor.matmul(
                            ps[:, 0, :], w_sb[:, c, :], xT_sb[:, c, sl],
                            start=(c == 0), stop=(c == 7),
                        )
                    if which == 0:
                        evict_q(n, ps[:, 0, :])
                    else:
                        evict_kv(n, ps[:, 0, :])

        # ---- attention + interleaved tails ----
        pending_tail = []

        def drain_one():
            if pending_tail:
                pending_tail.pop(0)()

        def make_tail(qc, ctx0, ctx1):
            qsl = slice(qc * QC, (qc + 1) * QC)
            st = {}

            def headchain(ctx_h, h, po):
                # raw ctx -> ctxT (frees the PSUM bank without waiting on the
                # reciprocal), then 1/den via integer-seed + one Newton step:
                #   r0 = bitcast(C - bits(den));  rr = (den*r0 - 2)*r0 = -1/den
                # broadcast with a minus-ones column restores the sign.
                nc.vector.tensor_copy(ctxT_sb[64 * h:64 * (h + 1), qsl],
                                      ctx_h[0:64, :])
                r0 = small.tile([128, QC], fp32, tag="r0", name=f"r0_{h}")
                nc.vector.tensor_scalar(
                    out=r0[64:65, :].bitcast(i32),
                    in0=ctx_h[64:65, :].bitcast(i32),
                    scalar1=-1.0, scalar2=C_RECIP, op0=MULT, op1=ADD,
                )
                tnw = small.tile([128, QC], fp32, tag="tnw", name=f"tnw{h}")
                nc.vector.tensor_mul(tnw[64:65, :], ctx_h[64:65, :],
                                     r0[64:65, :])
                rr = small.tile([128, QC], bf16, tag="rr", name=f"rr{h}")
                nc.vector.scalar_tensor_tensor(
                    out=rr[64:65, :], in0=tnw[64:65, :], scalar=2.0,
                    in1=r0[64:65, :], op0=SUB, op1=MULT,
                )
                nc.tensor.matmul(po[:, h, :], mones_sb[64:65, 0:128],
                                 rr[64:65, :], start=True, stop=True)

            def t1():
                po = psum.tile([128, GROUP, QC], fp32, tag="po", bufs=1,
                               name="po_rb")
                headchain(ctx0, 0, po)
                st["po_rb"] = po

            def t2():
                headchain(ctx1, 1, st["po_rb"])

            def t3():  # broadcasted recips -> normalize ctxT in place (bf16)
                po = st.pop("po_rb")
                rb0 = small.tile([128, QC], bf16, tag="rb", name="rb0")
                rb1 = small.tile([128, QC], bf16, tag="rb", name="rb1")
                nc.vector.tensor_copy(rb0, po[:, 0, :])
                nc.vector.tensor_mul(ctxT_sb[0:64, qsl],
                                     ctxT_sb[0:64, qsl], rb0[0:64, :])
                nc.vector.tensor_copy(rb1, po[:, 1, :])
                nc.vector.tensor_mul(ctxT_sb[64:128, qsl],
                                     ctxT_sb[64:128, qsl], rb1[64:128, :])

            def outproj(j):
                def f():
                    rc = 4 * qc + j
                    rsl = slice(rc * 128, (rc + 1) * 128)
                    po = psum.tile([128, GROUP, QC], fp32, tag="po", bufs=1,
                                   name=f"po_o{j}")
                    for e in range(2):
                        nc.tensor.matmul(
                            po[:, e, :], ctxT_sb[:, rsl],
                            woT_sb[:, e * QC:(e + 1) * QC],
                            start=True, stop=True,
                        )
                    ot = ost.tile([128, DIM], fp32, tag="ot", name=f"ot{j}")
                    nc.vector.tensor_copy(ot[:, 0:QC], po[:, 0, :])
                    nc.scalar.copy(ot[:, QC:DIM], po[:, 1, :])
                    nc.gpsimd.dma_start(out=out_d[rsl, :], in_=ot)
                return f

            return [t1, t2, t3, outproj(0), outproj(1), outproj(2), outproj(3)]

        def attention_qc(qc):
            nkt = 4 * qc + 4
            qsl = slice(qc * QC, (qc + 1) * QC)
            ctx0 = psum.tile([65, QC], fp32, tag="ctx", bufs=2, name="ctx0")
            ctx1 = psum.tile([65, QC], fp32, tag="ctx", bufs=2, name="ctx1")
            pend = None

            def emit_ctx(item):
                g0, pA, pB = item
                for i in range(GROUP):
                    kt = g0 + i
                    st_ = (kt == 0)
                    sp_ = (kt == nkt - 1)
                    nc.tensor.matmul(ctx0, vaug_sb[:, kt, 0:65], pA[:, i, :],
                                     start=st_, stop=sp_)
                    nc.tensor.matmul(ctx1, vaug_sb[:, kt, 0:65], pB[:, i, :],
                                     start=st_, stop=sp_)

            for gi, g0 in enumerate(range(0, nkt, GROUP)):
                psA = psum.tile([128, GROUP, QC], fp32, tag="s", bufs=2,
                                name="psA")
                psB = psum.tile([128, GROUP, QC], fp32, tag="s", bufs=2,
                                name="psB")
                ptA = ptp.tile([128, GROUP, QC], bf16, tag="ptA", name="ptA")
                ptB = ptp.tile([128, GROUP, QC], bf16, tag="ptB", name="ptB")
                for i in range(GROUP):
                    kt = g0 + i
                    ksl = slice(kt * 128, (kt + 1) * 128)
                    nc.tensor.matmul(psA[:, i, :], kds_sb[0:64, ksl],
                                     qT_sb[0:64, qsl], start=True, stop=True)
                    nc.tensor.matmul(psB[:, i, :], kds_sb[64:128, ksl],
                                     qT_sb[64:128, qsl], start=True, stop=True)
                nc.scalar.activation(ptA, psA, Exp, scale=0.125)
                if gi >= 7:
                    # head1 exp via DVE int-trick (bf16 bit pattern, ~2-3% p
                    # error, washed out by softmax normalization). Only in
                    # late groups, where no tail work contends for the DVE.
                    nc.vector.tensor_scalar(
                        out=ptB.bitcast(i16), in0=psB,
                        scalar1=K_EXP, scalar2=B_EXP, op0=MULT, op1=ADD,
                    )
                else:
                    nc.scalar.activation(ptB, psB, Exp, scale=0.125)
                for i in range(GROUP):
                    kt = g0 + i
                    r = kt - 4 * qc
                    if r >= 0:  # strip intersects the causal diagonal
                        for pt in (ptA, ptB):
                            if r >= 1:
                                nc.gpsimd.memset(pt[:, i, 0:128 * r], 0.0)
                            nc.gpsimd.tensor_mul(
                                pt[:, i, 128 * r:128 * (r + 1)],
                                pt[:, i, 128 * r:128 * (r + 1)],
                                mask_sb,
                            )
                drain_one()  # one tail stage of the previous q-chunk
                if pend is not None:
                    emit_ctx(pend)
                pend = (g0, ptA, ptB)
            emit_ctx(pend)

            for f in pending_tail:  # leftovers (early, short q-chunks)
                f()
            pending_tail.clear()
            pending_tail.extend(make_tail(qc, ctx0, ctx1))

        attention_qc(0)
        proj_rest()
        for qc in range(1, NQC):
            attention_qc(qc)
        for f in pending_tail:
            f()

        if debug:
            nc.sync.dma_start(out=dbg_qT[:], in_=qT_sb)
            nc.sync.dma_start(out=dbg_kds[:], in_=kds_sb)
            nc.sync.dma_start(out=dbg_vaug[:], in_=vaug_sb)
            nc.sync.dma_start(out=dbg_ctxT[:], in_=ctxT_sb)

    nc.compile()
    return nc


def _get_nc():
    if "nc" not in _CACHE:
        _CACHE["nc"] = _build_nc()
    return _CACHE["nc"]


def _prep_inputs(x, wq, wk, wv, wo):
    GS = NH // NKV
    x2 = np.asarray(x, np.float32).reshape(S, DIN)
    xT = np.ascontiguousarray(x2.T).astype(BF16)
    tri = (np.arange(128)[None, :] >= np.arange(128)[:, None]).astype(BF16)
    in_maps = []
    for c in range(NCORES):
        h0 = 2 * c
        g = h0 // GS
        wq_c = np.asarray(wq, np.float32)[h0 * HD:(h0 + 2) * HD, :]
        wkv_c = np.concatenate(
            [
                np.asarray(wk, np.float32)[g * HD:(g + 1) * HD, :],
                np.asarray(wv, np.float32)[g * HD:(g + 1) * HD, :],
            ],
            axis=0,
        )
        woT_c = np.asarray(wo, np.float32)[:, h0 * HD:(h0 + 2) * HD].T
        in_maps.append(
            {
                "xT": xT,
                "wqT": np.ascontiguousarray(wq_c.T).astype(BF16),
                "wkvT": np.ascontiguousarray(wkv_c.T).astype(BF16),
                "woT": np.ascontiguousarray(woT_c).astype(BF16),
                "trimask": tri,
            }
        )
    return in_maps


def _run(in_maps, trace=False):
    import sys
    if "/opt/trn_rl_repo" not in sys.path:
        sys.path.insert(0, "/opt/trn_rl_repo")
    from concourse.bass_utils import run_bass_kernel_spmd

    nc = _get_nc()
    res = run_bass_kernel_spmd(nc, in_maps, list(range(NCORES)), trace=trace)
    return res


def kernel(x, wq, wk, wv, wo):
    in_maps = _prep_inputs(x, wq, wk, wv, wo)
    res = _run(in_maps)
    parts = np.stack([np.asarray(r["out"], np.float32) for r in res.results])
    out = parts.sum(axis=0, dtype=np.float64).astype(np.float32)
    return out.reshape(1, S, DIM)
